# revision 10
# baseline (speedup 1.0000x reference)
"""BiLSTM-CRF loss kernel for Trainium2 (8 NeuronCores, Bass/Tile).

Strategy (v2)
-------------
Cores 0-3 run the FORWARD LSTM direction, cores 4-7 the BACKWARD direction
(fed time-reversed x), each over 16 of the 64 sequences.  Pair (c, c+4)
handles the same 16 sequences.

Per core, fully SBUF-resident pipeline (no DRAM roundtrips for gx / h):
  A) GX = (x @ W_ih^T + bias) * SCL  -- computed per 32-step slice into an
     SBUF ring, just-in-time ahead of the recurrence.  Emitted at low
     scheduler priority so its matmuls fill the PE gaps left by the
     recurrence's serial chain (also keeps the PE p-state high).
  B) LSTM recurrence at high scheduler priority.  W_hh is stored fp8-e4m3
     scaled by SCL=64 (4x faster LDWEIGHTS via fast-weight-load); the gx
     term is accumulated into PSUM with an identity-weight matmul so the
     activations read PSUM directly (scale=1/SCL folded into the act).
     Per-gate PSUM tiles (i+f, g, o-last) let sigmoid/tanh start while o's
     matmuls still run; tail is sig_o -> h-mul only.
  C) em partials per 16-step h chunk (tiny matmuls, fill gaps), masked
     into em0/em1 (bf16) and pair-AllReduced; bwd slot read back reversed.
  D) CRF: gold score via on-device one-hot (built on GpSimd during B);
     partition function via probability-domain scan, two interleaved
     8-sequence chains with staggered rescaling every R steps.
"""

import sys

sys.path.insert(0, "/opt/trn_rl_repo")

import numpy as np
import ml_dtypes
from contextlib import ExitStack

import concourse.bass as bass
import concourse.bacc as bacc
import concourse.tile as tile
import concourse.mybir as mybir

F32 = mybir.dt.float32
BF16 = mybir.dt.bfloat16
F8 = mybir.dt.float8e4
I32 = mybir.dt.int32
AFT = mybir.ActivationFunctionType
ALU = mybir.AluOpType
AXL = mybir.AxisListType

NCORES = 8
NPAIR = 4  # fwd cores 0..3, bwd cores 4..7

WHH_FP8 = True
SCL = 64.0 if WHH_FP8 else 1.0
WDT = F8 if WHH_FP8 else BF16
PRI_B = 2_000_000   # recurrence priority band
PRI_D = 1_000_000   # CRF-scan priority band


def build_program(b, S, E, HD, T, B_full, R=12, CH=16, TSL=32):
    KE = E // 128          # input-proj K tiles
    NH = HD // 128         # hidden K tiles
    NM = 4 * NH            # gate m-tiles (natural torch order i,f,g,o)
    SB = S * b
    W = NH * b             # h column width (kt-major, then batch)
    NSL = TSL * b          # columns per A slice
    NSLICE = S // TSL
    assert S % TSL == 0 and TSL % CH == 0 or CH % TSL == 0 or S % CH == 0

    nc = bacc.Bacc("TRN2", target_bir_lowering=False, debug=False,
                   num_devices=NCORES)

    # ---- I/O ----
    xT = nc.dram_tensor("xT", [KE, 128, SB], BF16, kind="ExternalInput")
    wihT = nc.dram_tensor("wihT", [KE, 128, 4 * HD], BF16, kind="ExternalInput")
    whhT = nc.dram_tensor("whhT", [NH, 128, 4 * HD], WDT, kind="ExternalInput")
    idenT = nc.dram_tensor("idenT", [128, 128], WDT, kind="ExternalInput")
    bias4 = nc.dram_tensor("bias4", [128, NM], F32, kind="ExternalInput")
    wtagT = nc.dram_tensor("wtagT", [NH, 128, T], BF16, kind="ExternalInput")
    tagb = nc.dram_tensor("tagb", [T, 1], F32, kind="ExternalInput")
    m0 = nc.dram_tensor("m0", [T, 1], F32, kind="ExternalInput")
    m1 = nc.dram_tensor("m1", [T, 1], F32, kind="ExternalInput")
    labT = nc.dram_tensor("labT", [S, b], I32, kind="ExternalInput")
    transm = nc.dram_tensor("transm", [T, T], F32, kind="ExternalInput")
    startv = nc.dram_tensor("startv", [T, 1], F32, kind="ExternalInput")
    endv = nc.dram_tensor("endv", [T, 1], F32, kind="ExternalInput")
    loss = nc.dram_tensor("loss", [1, 1], F32, kind="ExternalOutput")

    with tile.TileContext(nc) as tc, ExitStack() as top:
        dram = top.enter_context(tc.tile_pool(name="dram", bufs=1, space="DRAM"))
        emdb = dram.tile([2, T, SB], BF16)
        emdbo = dram.tile([2, T, SB], BF16)
        lossdb = dram.tile([1, 1], F32)
        lossout = dram.tile([1, 1], F32)

        persist = top.enter_context(tc.tile_pool(name="persist", bufs=1))
        whh_sb = persist.tile([128, NH * 4 * HD], WDT)
        nc.sync.dma_start(whh_sb[:], whhT[:])
        iden_sb = persist.tile([128, 128], WDT)
        nc.sync.dma_start(iden_sb[:], idenT[:])
        wih_sb = persist.tile([128, KE * 4 * HD], BF16)
        nc.sync.dma_start(wih_sb[:], wihT[:])
        bias_sb = persist.tile([128, NM], F32)
        nc.sync.dma_start(bias_sb[:], bias4[:])
        wtag_sb = persist.tile([128, NH * T], BF16)
        nc.sync.dma_start(wtag_sb[:], wtagT[:])
        tagb_sb = persist.tile([T, 1], F32)
        nc.sync.dma_start(tagb_sb[:], tagb[:])
        m0_sb = persist.tile([T, 1], F32)
        nc.sync.dma_start(m0_sb[:], m0[:])
        m1_sb = persist.tile([T, 1], F32)
        nc.sync.dma_start(m1_sb[:], m1[:])
        cst = persist.tile([T, T], F32)
        nc.sync.dma_start(cst[:], transm[:])
        st_sb = persist.tile([T, 1], F32)
        nc.sync.dma_start(st_sb[:], startv[:])
        en_sb = persist.tile([T, 1], F32)
        nc.sync.dma_start(en_sb[:], endv[:])
        em0 = persist.tile([T, SB], BF16)
        em1 = persist.tile([T, SB], BF16)
        onehot = persist.tile([T, SB], F32)
        ones9 = persist.tile([T, 1], F32)
        nc.vector.memset(ones9[:], 1.0)
        ones19 = persist.tile([1, T], F32)
        nc.vector.memset(ones19[:], 1.0)
        io9 = persist.tile([T, 1], I32)
        nc.gpsimd.iota(io9[:], pattern=[[0, 1]], base=0, channel_multiplier=1)
        io9f = persist.tile([T, 1], F32)
        nc.gpsimd.tensor_copy(io9f[:], io9[:])

        # one-hot labels on GpSimd (runs during phase B; only needs labT)
        with tc.tile_pool(name="labst", bufs=2) as lp:
            NLC = 8
            LC = SB // NLC
            labflat = labT[:].rearrange("s c -> (s c)")
            for k in range(NLC):
                labch = lp.tile([T, LC], I32, tag="labch")
                nc.sync.dma_start(
                    labch[:], labflat[None, k * LC:(k + 1) * LC]
                    .broadcast_to((T, LC)))
                labf = lp.tile([T, LC], F32, tag="labf")
                nc.gpsimd.tensor_copy(labf[:], labch[:])
                nc.gpsimd.tensor_scalar(onehot[:, k * LC:(k + 1) * LC],
                                        labf[:], io9f[:], None,
                                        op0=ALU.is_equal)

        # ---------------- phases A+B+C (SBUF-resident pipeline) ------------
        ab = ExitStack()
        gxp = ab.enter_context(tc.tile_pool(name="gxring", bufs=3))
        xp = ab.enter_context(tc.tile_pool(name="xring", bufs=3))
        hp = ab.enter_context(tc.tile_pool(name="hring", bufs=3))
        tp = ab.enter_context(tc.tile_pool(name="btmp", bufs=2))
        cp = ab.enter_context(tc.tile_pool(name="cstate", bufs=1))
        pa = ab.enter_context(tc.tile_pool(name="apsum", bufs=2, space="PSUM"))
        pif = ab.enter_context(tc.tile_pool(name="psif", bufs=1, space="PSUM"))
        pg = ab.enter_context(tc.tile_pool(name="psg", bufs=1, space="PSUM"))
        po = ab.enter_context(tc.tile_pool(name="pso", bufs=1, space="PSUM"))
        pc = ab.enter_context(tc.tile_pool(name="cpsum", bufs=2, space="PSUM"))

        c_sb = cp.tile([128, W], F32)

        gx_tiles = {}

        def a_slice(n):
            xt = xp.tile([128, KE * NSL], BF16, tag="xt")
            nc.sync.dma_start(xt[:], xT[:, :, n * NSL:(n + 1) * NSL])
            gx = gxp.tile([128, NM * NSL], BF16, tag="gx")
            gx_tiles[n] = gx
            for mm in range(NM):
                ps = pa.tile([128, NSL], F32)
                for ke in range(KE):
                    nc.tensor.matmul(
                        ps[:],
                        wih_sb[:, ke * 4 * HD + mm * 128:
                               ke * 4 * HD + (mm + 1) * 128],
                        xt[:, ke * NSL:(ke + 1) * NSL],
                        start=(ke == 0), stop=(ke == KE - 1))
                nc.vector.tensor_scalar(gx[:, mm * NSL:(mm + 1) * NSL], ps[:],
                                        bias_sb[:, mm:mm + 1], SCL,
                                        op0=ALU.add, op1=ALU.mult)

        a_slice(0)
        a_slice(1)

        hch = None
        h_prev = None
        for n in range(NSLICE):
            gx = gx_tiles[n]
            # per-step view: [p, mm, t_loc, b]
            gxv = gx[:].rearrange("p (m t c) -> p m t c", m=NM, t=TSL)
            for t_loc in range(TSL):
                t = n * TSL + t_loc
                toff = t % CH
                if toff == 0:
                    hch = hp.tile([128, CH * W], BF16, tag="hch")
                with tc.high_priority(offset=PRI_B):
                    if t == 0:
                        sig = tp.tile([128, 2 * W], F32, tag="sig")
                        nc.scalar.activation(
                            sig[:].rearrange("p (m c) -> p m c", m=2 * NH),
                            gxv[:, 0:2 * NH, 0, :], AFT.Sigmoid,
                            scale=1.0 / SCL)
                        tg = tp.tile([128, W], F32, tag="tg")
                        nc.scalar.activation(
                            tg[:].rearrange("p (m c) -> p m c", m=NH),
                            gxv[:, 2 * NH:3 * NH, 0, :], AFT.Tanh,
                            scale=1.0 / SCL)
                        so = tp.tile([128, W], F32, tag="so")
                        nc.scalar.activation(
                            so[:].rearrange("p (m c) -> p m c", m=NH),
                            gxv[:, 3 * NH:4 * NH, 0, :], AFT.Sigmoid,
                            scale=1.0 / SCL)
                        nc.vector.tensor_mul(c_sb[:], sig[:, 0:W], tg[:])
                    else:
                        ps_if = pif.tile([128, 2 * W], F32)
                        ps_g = pg.tile([128, W], F32)
                        ps_o = po.tile([128, W], F32)

                        def gate_mms(ps, mlo, mn, gslice):
                            # gx term first (start=True, no h dependency),
                            # then accumulate the W_hh @ h terms on top
                            nc.tensor.matmul(
                                ps[:].rearrange("p (m c) -> p m c", m=mn),
                                iden_sb[:],
                                gxv[:, mlo:mlo + mn, t_loc, :],
                                start=True, stop=False,
                                skip_group_check=True)
                            for ml in range(mn):
                                mm = mlo + ml
                                for kt in range(NH):
                                    nc.tensor.matmul(
                                        ps[:, ml * b:(ml + 1) * b],
                                        whh_sb[:, kt * 4 * HD + mm * 128:
                                               kt * 4 * HD + (mm + 1) * 128],
                                        h_prev[:, kt * b:(kt + 1) * b],
                                        start=False,
                                        stop=(ml == mn - 1 and kt == NH - 1),
                                        skip_group_check=True)

                        gate_mms(ps_if, 0, 2 * NH, None)
                        gate_mms(ps_g, 2 * NH, NH, None)
                        gate_mms(ps_o, 3 * NH, NH, None)

                        sig = tp.tile([128, 2 * W], F32, tag="sig")
                        nc.scalar.activation(sig[:], ps_if[:], AFT.Sigmoid,
                                             scale=1.0 / SCL)
                        tg = tp.tile([128, W], F32, tag="tg")
                        nc.scalar.activation(tg[:], ps_g[:], AFT.Tanh,
                                             scale=1.0 / SCL)
                        t1 = tp.tile([128, W], F32, tag="t1")
                        nc.vector.tensor_mul(t1[:], sig[:, W:2 * W], c_sb[:])
                        t2 = tp.tile([128, W], F32, tag="t2")
                        nc.vector.tensor_mul(t2[:], sig[:, 0:W], tg[:])
                        nc.vector.tensor_add(c_sb[:], t1[:], t2[:])
                        so = tp.tile([128, W], F32, tag="so")
                        nc.scalar.activation(so[:], ps_o[:], AFT.Sigmoid,
                                             scale=1.0 / SCL)
                    tanc = tp.tile([128, W], F32, tag="tanc")
                    nc.scalar.activation(tanc[:], c_sb[:], AFT.Tanh)
                    h_prev = hch[:, toff * W:(toff + 1) * W]
                    nc.vector.tensor_mul(h_prev, so[:], tanc[:])

                # phase C: em partial for the completed 16-step chunk
                if toff == CH - 1:
                    ch = t // CH
                    hv = hch[:].rearrange("p (t k c) -> p t k c", t=CH, k=NH)
                    psc = pc.tile([T, CH * b], F32)
                    for kt in range(NH):
                        nc.tensor.matmul(
                            psc[:].rearrange("p (t c) -> p t c", t=CH),
                            wtag_sb[:, kt * T:(kt + 1) * T],
                            hv[:, :, kt, :],
                            start=(kt == 0), stop=(kt == NH - 1))
                    nc.vector.tensor_scalar(
                        em0[:, ch * CH * b:(ch + 1) * CH * b], psc[:],
                        tagb_sb[:], m0_sb[:], op0=ALU.add, op1=ALU.mult)
                    nc.vector.tensor_scalar(
                        em1[:, ch * CH * b:(ch + 1) * CH * b], psc[:],
                        tagb_sb[:], m1_sb[:], op0=ALU.add, op1=ALU.mult)

            if n + 2 < NSLICE:
                a_slice(n + 2)

        # pair exchange of em partials (fwd slot 0, bwd slot 1, bf16)
        nc.sync.dma_start(emdb[0], em0[:])
        nc.sync.dma_start(emdb[1], em1[:])
        nc.gpsimd.collective_compute(
            "AllReduce", ALU.add,
            replica_groups=[[c, c + NPAIR] for c in range(NPAIR)],
            ins=[emdb.opt()], outs=[emdbo.opt()])
        ab.close()

        # ---------------- Phase D: CRF ----------------
        big = top.enter_context(tc.tile_pool(name="crfbig", bufs=1))
        sp = top.enter_context(tc.tile_pool(name="crftmp", bufs=2))
        ap_ = top.enter_context(tc.tile_pool(name="alphas", bufs=2))
        pp = top.enter_context(tc.tile_pool(name="dpsum", bufs=2, space="PSUM"))
        pt = top.enter_context(tc.tile_pool(name="tpsum", bufs=1, space="PSUM"))
        pps = top.enter_context(tc.tile_pool(name="spsum", bufs=1, space="PSUM"))

        # reuse em0/em1 (dead after the DMA-out) as collective staging
        nc.sync.dma_start(em0[:], emdbo[0])
        nc.sync.dma_start(
            em1[:], emdbo[1].rearrange("j (t c) -> j t c", t=S)[:, ::-1, :])
        em_full = big.tile([T, SB], F32, tag="emfull")
        eem = big.tile([T, SB], F32, tag="eem")
        NEC = 4
        EC = SB // NEC
        for k in range(NEC):
            nc.vector.tensor_add(em_full[:, k * EC:(k + 1) * EC],
                                 em0[:, k * EC:(k + 1) * EC],
                                 em1[:, k * EC:(k + 1) * EC])
            nc.scalar.activation(eem[:, k * EC:(k + 1) * EC],
                                 em_full[:, k * EC:(k + 1) * EC], AFT.Exp)

        Em = sp.tile([T, T], F32, tag="Em")
        nc.scalar.activation(Em[:], cst[:], AFT.Exp)
        es = sp.tile([T, 1], F32, tag="es")
        nc.scalar.activation(es[:], st_sb[:], AFT.Exp)
        ee = sp.tile([T, 1], F32, tag="ee")
        nc.scalar.activation(ee[:], en_sb[:], AFT.Exp)

        # --- partition function: two interleaved 8-seq chains ---
        b2 = b // 2
        logacc = [sp.tile([1, b2], F32, tag=f"lacc{c}", name=f"lacc{c}")
                  for c in range(2)]
        alpha = [None, None]
        with tc.high_priority(offset=PRI_D):
            for c in range(2):
                nc.vector.memset(logacc[c][:], 0.0)
                al = ap_.tile([T, b2], F32, tag=f"al{c}")
                nc.vector.tensor_scalar_mul(
                    al[:], eem[:, c * b2:c * b2 + b2], es[:])
                alpha[c] = al

            def rescale(c):
                ssum = pps.tile([1, b2], F32, tag="ssum")
                nc.tensor.matmul(ssum[:], ones9[:], alpha[c][:],
                                 start=True, stop=True)
                ls = sp.tile([1, b2], F32, tag=f"ls{c}")
                nc.scalar.activation(ls[:], ssum[:], AFT.Ln)
                nc.vector.tensor_add(logacc[c][:], logacc[c][:], ls[:])
                rc = sp.tile([1, b2], F32, tag=f"rc{c}")
                nc.vector.reciprocal(rc[:], ssum[:])
                bc = pps.tile([T, b2], F32, tag="bc")
                nc.tensor.matmul(bc[:], ones19[:], rc[:],
                                 start=True, stop=True)
                a2 = ap_.tile([T, b2], F32, tag=f"al{c}")
                nc.vector.tensor_mul(a2[:], alpha[c][:], bc[:])
                alpha[c] = a2

            for t in range(1, S):
                for c in range(2):
                    aps = pp.tile([T, b2], F32, tag=f"aps{c}")
                    nc.tensor.matmul(aps[:], Em[:], alpha[c][:],
                                     start=True, stop=True)
                    al = ap_.tile([T, b2], F32, tag=f"al{c}")
                    nc.vector.tensor_mul(
                        al[:], aps[:], eem[:, t * b + c * b2:t * b + c * b2 + b2])
                    alpha[c] = al
                for c in range(2):
                    if t % R == (R // 2) * c or t == S - 1:
                        rescale(c)

        # --- numerator (gold-path score); fills gaps during the scan ---
        acc = sp.tile([T, b], F32, tag="acc")
        nc.vector.memset(acc[:], 0.0)
        NGC = 8
        GC = SB // NGC
        for k in range(NGC):
            gm = sp.tile([T, GC], F32, tag="gm")
            nc.vector.tensor_mul(gm[:], onehot[:, k * GC:(k + 1) * GC],
                                 em_full[:, k * GC:(k + 1) * GC])
            gr = sp.tile([T, b], F32, tag="gr")
            nc.vector.tensor_reduce(
                gr[:], gm[:].rearrange("j (t c) -> j c t", c=b),
                op=ALU.add, axis=AXL.X)
            nc.vector.tensor_add(acc[:], acc[:], gr[:])
        stsc = sp.tile([T, b], F32, tag="stsc")
        nc.vector.tensor_scalar_mul(stsc[:], onehot[:, 0:b], st_sb[:])
        nc.vector.tensor_add(acc[:], acc[:], stsc[:])
        ensc = sp.tile([T, b], F32, tag="ensc")
        nc.vector.tensor_scalar_mul(ensc[:], onehot[:, (S - 1) * b:S * b],
                                    en_sb[:])
        nc.vector.tensor_add(acc[:], acc[:], ensc[:])
        for tc0 in range(0, S - 1, 32):
            tn = min(32, S - 1 - tc0)
            thp = pt.tile([T, 32 * b], F32, tag="thp")
            nc.tensor.matmul(thp[:, 0:tn * b], cst[:],
                             onehot[:, tc0 * b:(tc0 + tn) * b],
                             start=True, stop=True)
            v = sp.tile([T, 32 * b], F32, tag="v")
            nc.vector.tensor_mul(v[:, 0:tn * b], thp[:, 0:tn * b],
                                 onehot[:, (tc0 + 1) * b:(tc0 + 1 + tn) * b])
            vr = sp.tile([T, b], F32, tag="vr")
            nc.vector.tensor_reduce(
                vr[:], v[:, 0:tn * b].rearrange("j (t c) -> j c t", c=b),
                op=ALU.add, axis=AXL.X)
            nc.vector.tensor_add(acc[:], acc[:], vr[:])
        nump = pt.tile([1, b], F32, tag="nump")
        nc.tensor.matmul(nump[:], ones9[:], acc[:], start=True, stop=True)
        num_sb = sp.tile([1, b], F32, tag="num")
        nc.vector.tensor_copy(num_sb[:], nump[:])

        # --- logZ + loss ---
        logz = sp.tile([1, b], F32, tag="logz")
        for c in range(2):
            zp = pp.tile([1, b2], F32, tag=f"aps{c}")
            nc.tensor.matmul(zp[:], ee[:], alpha[c][:], start=True, stop=True)
            lz = sp.tile([1, b2], F32, tag=f"lz{c}")
            nc.scalar.activation(lz[:], zp[:], AFT.Ln)
            nc.vector.tensor_add(logz[:, c * b2:(c + 1) * b2], lz[:],
                                 logacc[c][:])
        lv = sp.tile([1, b], F32, tag="lv")
        nc.vector.tensor_sub(lv[:], num_sb[:], logz[:])
        tot = sp.tile([1, 1], F32, tag="tot")
        nc.vector.tensor_reduce(tot[:], lv[:], op=ALU.add, axis=AXL.X)
        sc = sp.tile([1, 1], F32, tag="sc")
        nc.vector.tensor_scalar_mul(sc[:], tot[:], -1.0 / (2.0 * B_full))
        nc.sync.dma_start(lossdb[:], sc[:])
        nc.gpsimd.collective_compute(
            "AllReduce", ALU.add,
            replica_groups=[list(range(NCORES))],
            ins=[lossdb.opt()], outs=[lossout.opt()])
        lf = sp.tile([1, 1], F32, tag="lf")
        nc.sync.dma_start(lf[:], lossout[:])
        nc.sync.dma_start(loss[:], lf[:])

    nc.compile()
    return nc


# ---------------------------------------------------------------------------
# host-side sharding
# ---------------------------------------------------------------------------

def shard_inputs(inputs, b, S, E, HD, T):
    KE, NH = E // 128, HD // 128
    bf = ml_dtypes.bfloat16
    f8 = ml_dtypes.float8_e4m3
    wdt = f8 if WHH_FP8 else bf
    x = np.asarray(inputs["x"], np.float32)
    labels = np.asarray(inputs["labels"]).astype(np.int32)
    trans = np.asarray(inputs["transitions"], np.float32)
    startv = np.asarray(inputs["start_trans"], np.float32).reshape(T, 1)
    endv = np.asarray(inputs["end_trans"], np.float32).reshape(T, 1)
    Wtag = np.asarray(inputs["W_tag"], np.float32)
    btag = np.asarray(inputs["b_tag"], np.float32).reshape(T, 1)
    iden = np.eye(128, dtype=np.float32).astype(wdt)

    per_dir = {}
    for d, sfx in enumerate(("f", "b")):
        Wih = np.asarray(inputs[f"W_ih_{sfx}"], np.float32)
        Whh = np.asarray(inputs[f"W_hh_{sfx}"], np.float32)
        bias = (np.asarray(inputs[f"b_ih_{sfx}"], np.float32)
                + np.asarray(inputs[f"b_hh_{sfx}"], np.float32))
        per_dir[d] = dict(
            wihT=np.ascontiguousarray(
                Wih.T.reshape(KE, 128, 4 * HD)).astype(bf),
            whhT=np.ascontiguousarray(
                (Whh * SCL).T.reshape(NH, 128, 4 * HD)).astype(wdt),
            bias4=np.ascontiguousarray(
                bias.reshape(4 * NH, 128).T).astype(np.float32),
            wtagT=np.ascontiguousarray(
                Wtag[:, d * HD:(d + 1) * HD].T.reshape(NH, 128, T)).astype(bf),
            tagb=btag if d == 0 else np.zeros_like(btag),
            m0=np.full((T, 1), 1.0 - d, np.float32),
            m1=np.full((T, 1), float(d), np.float32),
        )

    in_maps = []
    for c in range(NCORES):
        d = c // NPAIR                      # 0 fwd, 1 bwd
        g = c % NPAIR                       # batch group
        xs = x[g * b:(g + 1) * b]           # (b, S, E)
        if d == 1:
            xs = xs[:, ::-1, :]
        xTc = np.ascontiguousarray(xs.transpose(2, 1, 0).reshape(KE, 128, S * b)
                                   ).astype(bf)
        m = dict(per_dir[d])
        m["xT"] = xTc
        m["idenT"] = iden
        m["labT"] = np.ascontiguousarray(labels[g * b:(g + 1) * b].T)
        m["transm"] = trans
        m["startv"] = startv
        m["endv"] = endv
        in_maps.append(m)
    return in_maps


# ---------------------------------------------------------------------------
# entry point
# ---------------------------------------------------------------------------

_B, _S, _E, _HD, _T = 64, 512, 1024, 512, 9
_cache = {}


def _get_program():
    if "nc" not in _cache:
        _cache["nc"] = build_program(_B // NPAIR, _S, _E, _HD, _T, _B)
    return _cache["nc"]


def kernel(**inputs) -> np.ndarray:
    from concourse.bass_utils import run_bass_kernel_spmd
    nc = _get_program()
    in_maps = shard_inputs(inputs, _B // NPAIR, _S, _E, _HD, _T)
    res = run_bass_kernel_spmd(nc, in_maps, list(range(NCORES)))
    out = np.asarray(res.results[0]["loss"], np.float32).reshape(())
    return out


# revision 23
# speedup vs baseline: 1.0964x; 1.0964x over previous
"""BiLSTM-CRF loss kernel for Trainium2 (8 NeuronCores, Bass/Tile).

Strategy (v2)
-------------
Cores 0-3 run the FORWARD LSTM direction, cores 4-7 the BACKWARD direction
(fed time-reversed x), each over 16 of the 64 sequences.  Pair (c, c+4)
handles the same 16 sequences.

Per core, fully SBUF-resident pipeline (no DRAM roundtrips for gx / h):
  A) GX = (x @ W_ih^T + bias) * SCL  -- computed per 32-step slice into an
     SBUF ring, just-in-time ahead of the recurrence.  Emitted at low
     scheduler priority so its matmuls fill the PE gaps left by the
     recurrence's serial chain (also keeps the PE p-state high).
  B) LSTM recurrence at high scheduler priority.  W_hh is stored fp8-e4m3
     scaled by SCL=64 (4x faster LDWEIGHTS via fast-weight-load); the gx
     term is accumulated into PSUM with an identity-weight matmul so the
     activations read PSUM directly (scale=1/SCL folded into the act).
     Per-gate PSUM tiles (i+f, g, o-last) let sigmoid/tanh start while o's
     matmuls still run; tail is sig_o -> h-mul only.
  C) em partials per 16-step h chunk (tiny matmuls, fill gaps), masked
     into em0/em1 (bf16) and pair-AllReduced; bwd slot read back reversed.
  D) CRF: gold score via on-device one-hot (built on GpSimd during B);
     partition function via probability-domain scan, two interleaved
     8-sequence chains with staggered rescaling every R steps.
"""

import sys

sys.path.insert(0, "/opt/trn_rl_repo")

import numpy as np
import ml_dtypes
from contextlib import ExitStack

import concourse.bass as bass
import concourse.bacc as bacc
import concourse.tile as tile
import concourse.mybir as mybir

F32 = mybir.dt.float32
BF16 = mybir.dt.bfloat16
F8 = mybir.dt.float8e4
I32 = mybir.dt.int32
AFT = mybir.ActivationFunctionType
ALU = mybir.AluOpType
AXL = mybir.AxisListType

NCORES = 8
NPAIR = 4  # fwd cores 0..3, bwd cores 4..7

WHH_FP8 = True
SCL = 64.0 if WHH_FP8 else 1.0
WDT = F8 if WHH_FP8 else BF16
PRI_B = 2_000_000   # recurrence priority band
PRI_D = 1_000_000   # CRF-scan priority band


def build_program(b, S, E, HD, T, B_full, R=12, CH=16, TSL=32):
    KE = E // 128          # input-proj K tiles
    NH = HD // 128         # hidden K tiles
    NM = 4 * NH            # gate m-tiles (natural torch order i,f,g,o)
    SB = S * b
    W = NH * b             # h column width (kt-major, then batch)
    NSL = TSL * b          # columns per A slice
    NSLICE = S // TSL
    assert S % TSL == 0 and TSL % CH == 0 or CH % TSL == 0 or S % CH == 0

    nc = bacc.Bacc("TRN2", target_bir_lowering=False, debug=False,
                   num_devices=NCORES)

    # ---- I/O ----
    xT = nc.dram_tensor("xT", [KE, 128, SB], BF16, kind="ExternalInput")
    wihT = nc.dram_tensor("wihT", [KE, 128, 4 * HD], BF16, kind="ExternalInput")
    whhT = nc.dram_tensor("whhT", [NH, 128, 4 * HD], WDT, kind="ExternalInput")
    idenT = nc.dram_tensor("idenT", [128, 128], WDT, kind="ExternalInput")
    bias4 = nc.dram_tensor("bias4", [128, NM], F32, kind="ExternalInput")
    wtagT = nc.dram_tensor("wtagT", [NH, 128, T], BF16, kind="ExternalInput")
    tagb = nc.dram_tensor("tagb", [T, 1], F32, kind="ExternalInput")
    m0 = nc.dram_tensor("m0", [T, 1], F32, kind="ExternalInput")
    m1 = nc.dram_tensor("m1", [T, 1], F32, kind="ExternalInput")
    onehotI = nc.dram_tensor("onehotI", [T, SB], F32, kind="ExternalInput")
    goldc = nc.dram_tensor("goldc", [1, b], F32, kind="ExternalInput")
    transm = nc.dram_tensor("transm", [T, T], F32, kind="ExternalInput")
    startv = nc.dram_tensor("startv", [T, 1], F32, kind="ExternalInput")
    endv = nc.dram_tensor("endv", [T, 1], F32, kind="ExternalInput")
    loss = nc.dram_tensor("loss", [1, 1], F32, kind="ExternalOutput")

    with tile.TileContext(nc) as tc, ExitStack() as top:
        dram = top.enter_context(tc.tile_pool(name="dram", bufs=1, space="DRAM"))
        SH = SB // 2
        emdbA = dram.tile([2, T, SH], BF16)
        emdboA = dram.tile([2, T, SH], BF16)
        emdbB = dram.tile([2, T, SH], BF16)
        emdboB = dram.tile([2, T, SH], BF16)
        lossdb = dram.tile([1, 1], F32)
        lossout = dram.tile([1, 1], F32)

        persist = top.enter_context(tc.tile_pool(name="persist", bufs=1))
        whh_sb = persist.tile([128, NH * 4 * HD], WDT)
        nc.sync.dma_start(whh_sb[:], whhT[:])
        iden_sb = persist.tile([128, 128], WDT)
        nc.sync.dma_start(iden_sb[:], idenT[:])
        wih_sb = persist.tile([128, KE * 4 * HD], BF16)
        nc.sync.dma_start(wih_sb[:], wihT[:])
        bias_sb = persist.tile([128, NM], F32)
        nc.sync.dma_start(bias_sb[:], bias4[:])
        wtag_sb = persist.tile([128, NH * T], BF16)
        nc.sync.dma_start(wtag_sb[:], wtagT[:])
        tagb_sb = persist.tile([T, 1], F32)
        nc.sync.dma_start(tagb_sb[:], tagb[:])
        m0_sb = persist.tile([T, 1], F32)
        nc.sync.dma_start(m0_sb[:], m0[:])
        m1_sb = persist.tile([T, 1], F32)
        nc.sync.dma_start(m1_sb[:], m1[:])
        cst = persist.tile([T, T], F32)
        nc.sync.dma_start(cst[:], transm[:])
        st_sb = persist.tile([T, 1], F32)
        nc.sync.dma_start(st_sb[:], startv[:])
        en_sb = persist.tile([T, 1], F32)
        nc.sync.dma_start(en_sb[:], endv[:])
        em0 = persist.tile([T, SB], BF16)
        em1 = persist.tile([T, SB], BF16)
        onehot = persist.tile([T, SB], F32)
        nc.sync.dma_start(onehot[:], onehotI[:])
        goldc_sb = persist.tile([1, b], F32)
        nc.sync.dma_start(goldc_sb[:], goldc[:])
        ones9 = persist.tile([T, 1], F32)
        nc.vector.memset(ones9[:], 1.0)
        ones19 = persist.tile([1, T], F32)
        nc.vector.memset(ones19[:], 1.0)

        # ---------------- phases A+B+C (SBUF-resident pipeline) ------------
        ab = ExitStack()
        gxp = ab.enter_context(tc.tile_pool(name="gxring", bufs=3))
        xp = ab.enter_context(tc.tile_pool(name="xring", bufs=3))
        hp = ab.enter_context(tc.tile_pool(name="hring", bufs=3))
        tp = ab.enter_context(tc.tile_pool(name="btmp", bufs=2))
        cp = ab.enter_context(tc.tile_pool(name="cstate", bufs=1))
        pa = ab.enter_context(tc.tile_pool(name="apsum", bufs=2, space="PSUM"))
        pif = ab.enter_context(tc.tile_pool(name="psif", bufs=1, space="PSUM"))
        pg = ab.enter_context(tc.tile_pool(name="psg", bufs=1, space="PSUM"))
        po = ab.enter_context(tc.tile_pool(name="pso", bufs=1, space="PSUM"))
        pc = ab.enter_context(tc.tile_pool(name="cpsum", bufs=2, space="PSUM"))

        c_sb = cp.tile([128, W], BF16)

        gx_tiles = {}

        def a_slice(n):
            xt = xp.tile([128, KE * NSL], BF16, tag="xt")
            nc.sync.dma_start(xt[:], xT[:, :, n * NSL:(n + 1) * NSL])
            gx = gxp.tile([128, NM * NSL], BF16, tag="gx")
            gx_tiles[n] = gx
            for mm in range(NM):
                ps = pa.tile([128, NSL], F32)
                for ke in range(KE):
                    nc.tensor.matmul(
                        ps[:],
                        wih_sb[:, ke * 4 * HD + mm * 128:
                               ke * 4 * HD + (mm + 1) * 128],
                        xt[:, ke * NSL:(ke + 1) * NSL],
                        start=(ke == 0), stop=(ke == KE - 1))
                nc.vector.tensor_scalar(gx[:, mm * NSL:(mm + 1) * NSL], ps[:],
                                        bias_sb[:, mm:mm + 1], SCL,
                                        op0=ALU.add, op1=ALU.mult)

        a_slice(0)
        a_slice(1)

        hch = None
        h_prev = None
        for n in range(NSLICE):
            gx = gx_tiles[n]
            # per-step view: [p, mm, t_loc, b]
            gxv = gx[:].rearrange("p (m t c) -> p m t c", m=NM, t=TSL)
            for t_loc in range(TSL):
                t = n * TSL + t_loc
                toff = t % CH
                if toff == 0:
                    hch = hp.tile([128, CH * W], BF16, tag="hch")
                with tc.high_priority(offset=PRI_B):
                    if t == 0:
                        sig = tp.tile([128, 2 * W], BF16, tag="sig")
                        nc.scalar.activation(
                            sig[:].rearrange("p (m c) -> p m c", m=2 * NH),
                            gxv[:, 0:2 * NH, 0, :], AFT.Sigmoid,
                            scale=1.0 / SCL)
                        tg = tp.tile([128, W], BF16, tag="tg")
                        nc.scalar.activation(
                            tg[:].rearrange("p (m c) -> p m c", m=NH),
                            gxv[:, 2 * NH:3 * NH, 0, :], AFT.Tanh,
                            scale=1.0 / SCL)
                        so = tp.tile([128, W], BF16, tag="so")
                        nc.scalar.activation(
                            so[:].rearrange("p (m c) -> p m c", m=NH),
                            gxv[:, 3 * NH:4 * NH, 0, :], AFT.Sigmoid,
                            scale=1.0 / SCL)
                        nc.vector.tensor_mul(c_sb[:], sig[:, 0:W], tg[:])
                    else:
                        ps_if = pif.tile([128, 2 * W], F32)
                        ps_g = pg.tile([128, W], F32)
                        ps_o = po.tile([128, W], F32)

                        def gate_mms(ps, mlo, mn, gslice):
                            # gx term first (start=True, no h dependency),
                            # then accumulate the W_hh @ h terms on top
                            nc.tensor.matmul(
                                ps[:].rearrange("p (m c) -> p m c", m=mn),
                                iden_sb[:],
                                gxv[:, mlo:mlo + mn, t_loc, :],
                                start=True, stop=False,
                                skip_group_check=True)
                            for ml in range(mn):
                                mm = mlo + ml
                                for kt in range(NH):
                                    nc.tensor.matmul(
                                        ps[:, ml * b:(ml + 1) * b],
                                        whh_sb[:, kt * 4 * HD + mm * 128:
                                               kt * 4 * HD + (mm + 1) * 128],
                                        h_prev[:, kt * b:(kt + 1) * b],
                                        start=False,
                                        stop=(ml == mn - 1 and kt == NH - 1),
                                        skip_group_check=True)

                        gate_mms(ps_if, 0, 2 * NH, None)
                        gate_mms(ps_g, 2 * NH, NH, None)
                        gate_mms(ps_o, 3 * NH, NH, None)

                        sig = tp.tile([128, 2 * W], BF16, tag="sig")
                        nc.scalar.activation(sig[:], ps_if[:], AFT.Sigmoid,
                                             scale=1.0 / SCL)
                        tg = tp.tile([128, W], BF16, tag="tg")
                        nc.scalar.activation(tg[:], ps_g[:], AFT.Tanh,
                                             scale=1.0 / SCL)
                        t1 = tp.tile([128, W], BF16, tag="t1")
                        nc.vector.tensor_mul(t1[:], sig[:, W:2 * W], c_sb[:])
                        t2 = tp.tile([128, W], BF16, tag="t2")
                        nc.vector.tensor_mul(t2[:], sig[:, 0:W], tg[:])
                        nc.vector.tensor_add(c_sb[:], t1[:], t2[:])
                        so = tp.tile([128, W], BF16, tag="so")
                        nc.scalar.activation(so[:], ps_o[:], AFT.Sigmoid,
                                             scale=1.0 / SCL)
                    tanc = tp.tile([128, W], BF16, tag="tanc")
                    nc.scalar.activation(tanc[:], c_sb[:], AFT.Tanh)
                    h_prev = hch[:, toff * W:(toff + 1) * W]
                    nc.vector.tensor_mul(h_prev, so[:], tanc[:])

                # phase C: em partial for the completed 16-step chunk
                if toff == CH - 1:
                    ch = t // CH
                    hv = hch[:].rearrange("p (t k c) -> p t k c", t=CH, k=NH)
                    psc = pc.tile([T, CH * b], F32)
                    for kt in range(NH):
                        nc.tensor.matmul(
                            psc[:].rearrange("p (t c) -> p t c", t=CH),
                            wtag_sb[:, kt * T:(kt + 1) * T],
                            hv[:, :, kt, :],
                            start=(kt == 0), stop=(kt == NH - 1))
                    nc.vector.tensor_scalar(
                        em0[:, ch * CH * b:(ch + 1) * CH * b], psc[:],
                        tagb_sb[:], m0_sb[:], op0=ALU.add, op1=ALU.mult)
                    nc.vector.tensor_scalar(
                        em1[:, ch * CH * b:(ch + 1) * CH * b], psc[:],
                        tagb_sb[:], m1_sb[:], op0=ALU.add, op1=ALU.mult)

            if n + 2 < NSLICE:
                a_slice(n + 2)
            if n == NSLICE // 2 - 1:
                # first-half em partials exchange, overlapped with B's 2nd half
                nc.sync.dma_start(emdbA[0], em0[:, 0:SH])
                nc.sync.dma_start(emdbA[1], em1[:, 0:SH])
                nc.gpsimd.collective_compute(
                    "AllReduce", ALU.add,
                    replica_groups=[[c, c + NPAIR] for c in range(NPAIR)],
                    ins=[emdbA.opt()], outs=[emdboA.opt()])

        nc.sync.dma_start(emdbB[0], em0[:, SH:SB])
        nc.sync.dma_start(emdbB[1], em1[:, SH:SB])
        nc.gpsimd.collective_compute(
            "AllReduce", ALU.add,
            replica_groups=[[c, c + NPAIR] for c in range(NPAIR)],
            ins=[emdbB.opt()], outs=[emdboB.opt()])
        ab.close()

        # ---------------- Phase D: CRF ----------------
        big = top.enter_context(tc.tile_pool(name="crfbig", bufs=1))
        sp = top.enter_context(tc.tile_pool(name="crftmp", bufs=2))
        ap_ = top.enter_context(tc.tile_pool(name="alphas", bufs=2))
        pp = top.enter_context(tc.tile_pool(name="dpsum", bufs=1, space="PSUM"))
        pt = top.enter_context(tc.tile_pool(name="tpsum", bufs=1, space="PSUM"))
        pps = top.enter_context(tc.tile_pool(name="spsum", bufs=1, space="PSUM"))

        # reuse em0/em1 (dead after the DMA-out) as collective staging.
        # bwd slot is read back time-reversed; the global reversal maps
        # half A <-> half B with an intra-half reversal.
        S2 = S // 2
        nc.sync.dma_start(em0[:, 0:SH], emdboA[0])
        nc.sync.dma_start(em0[:, SH:SB], emdboB[0])
        nc.sync.dma_start(
            em1[:, 0:SH],
            emdboB[1].rearrange("j (t c) -> j t c", t=S2)[:, ::-1, :])
        nc.sync.dma_start(
            em1[:, SH:SB],
            emdboA[1].rearrange("j (t c) -> j t c", t=S2)[:, ::-1, :])
        em_full = big.tile([T, SB], F32, tag="emfull")
        eem = big.tile([T, SB], F32, tag="eem")
        NEC = 4
        EC = SB // NEC
        for k in range(NEC):
            nc.vector.tensor_add(em_full[:, k * EC:(k + 1) * EC],
                                 em0[:, k * EC:(k + 1) * EC],
                                 em1[:, k * EC:(k + 1) * EC])
            nc.scalar.activation(eem[:, k * EC:(k + 1) * EC],
                                 em_full[:, k * EC:(k + 1) * EC], AFT.Exp)

        Em = sp.tile([T, T], F32, tag="Em")
        nc.scalar.activation(Em[:], cst[:], AFT.Exp)
        es = sp.tile([T, 1], F32, tag="es")
        nc.scalar.activation(es[:], st_sb[:], AFT.Exp)
        ee = sp.tile([T, 1], F32, tag="ee")
        nc.scalar.activation(ee[:], en_sb[:], AFT.Exp)

        # --- partition function: three interleaved batch chains ---
        CH3 = [(0, 6), (6, 11), (11, 16)]
        NCH3 = len(CH3)
        logacc = [sp.tile([1, hi - lo], F32, tag=f"lacc{c}", name=f"lacc{c}")
                  for c, (lo, hi) in enumerate(CH3)]
        alpha = [None] * NCH3
        with tc.high_priority(offset=PRI_D):
            for c, (lo, hi) in enumerate(CH3):
                nc.vector.memset(logacc[c][:], 0.0)
                al = ap_.tile([T, hi - lo], F32, tag=f"al{c}", name=f"al{c}")
                nc.vector.tensor_scalar_mul(al[:], eem[:, lo:hi], es[:])
                alpha[c] = al

            def rescale(c):
                lo, hi = CH3[c]
                w = hi - lo
                ssum = pps.tile([1, w], F32, tag="ssum")
                nc.tensor.matmul(ssum[:], ones9[:], alpha[c][:],
                                 start=True, stop=True)
                ls = sp.tile([1, w], F32, tag=f"ls{c}", name=f"ls{c}")
                nc.scalar.activation(ls[:], ssum[:], AFT.Ln)
                nc.vector.tensor_add(logacc[c][:], logacc[c][:], ls[:])
                rc = sp.tile([1, w], F32, tag=f"rc{c}", name=f"rc{c}")
                nc.vector.reciprocal(rc[:], ssum[:])
                bc = pps.tile([T, w], F32, tag="bc")
                nc.tensor.matmul(bc[:], ones19[:], rc[:],
                                 start=True, stop=True)
                a2 = ap_.tile([T, w], F32, tag=f"al{c}", name=f"al{c}b")
                nc.vector.tensor_mul(a2[:], alpha[c][:], bc[:])
                alpha[c] = a2

            for t in range(1, S):
                for c, (lo, hi) in enumerate(CH3):
                    aps = pp.tile([T, hi - lo], F32, tag=f"aps{c}",
                                  name=f"aps{c}")
                    nc.tensor.matmul(aps[:], Em[:], alpha[c][:],
                                     start=True, stop=True)
                    al = ap_.tile([T, hi - lo], F32, tag=f"al{c}",
                                  name=f"al{c}c")
                    nc.vector.tensor_mul(
                        al[:], aps[:], eem[:, t * b + lo:t * b + hi])
                    alpha[c] = al
                for c in range(NCH3):
                    if t % R == 4 * c or t == S - 1:
                        rescale(c)

        # --- numerator (gold-path score); fills gaps during the scan ---
        acc = sp.tile([T, b], F32, tag="acc")
        nc.vector.memset(acc[:], 0.0)
        NGC = 8
        GC = SB // NGC
        for k in range(NGC):
            gm = sp.tile([T, GC], F32, tag="gm")
            nc.vector.tensor_mul(gm[:], onehot[:, k * GC:(k + 1) * GC],
                                 em_full[:, k * GC:(k + 1) * GC])
            gr = sp.tile([T, b], F32, tag="gr")
            nc.vector.tensor_reduce(
                gr[:], gm[:].rearrange("j (t c) -> j c t", c=b),
                op=ALU.add, axis=AXL.X)
            nc.vector.tensor_add(acc[:], acc[:], gr[:])
        # start/end/transition gold scores are label-only -> host (goldc)
        nump = pt.tile([1, b], F32, tag="nump")
        nc.tensor.matmul(nump[:], ones9[:], acc[:], start=True, stop=True)
        num_sb = sp.tile([1, b], F32, tag="num")
        nc.vector.tensor_add(num_sb[:], nump[:], goldc_sb[:])

        # --- logZ + loss ---
        logz = sp.tile([1, b], F32, tag="logz")
        for c, (lo, hi) in enumerate(CH3):
            zp = pp.tile([1, hi - lo], F32, tag=f"aps{c}", name=f"zp{c}")
            nc.tensor.matmul(zp[:], ee[:], alpha[c][:], start=True, stop=True)
            lz = sp.tile([1, hi - lo], F32, tag=f"lz{c}", name=f"lz{c}")
            nc.scalar.activation(lz[:], zp[:], AFT.Ln)
            nc.vector.tensor_add(logz[:, lo:hi], lz[:], logacc[c][:])
        lv = sp.tile([1, b], F32, tag="lv")
        nc.vector.tensor_sub(lv[:], num_sb[:], logz[:])
        tot = sp.tile([1, 1], F32, tag="tot")
        nc.vector.tensor_reduce(tot[:], lv[:], op=ALU.add, axis=AXL.X)
        sc = sp.tile([1, 1], F32, tag="sc")
        nc.vector.tensor_scalar_mul(sc[:], tot[:], -1.0 / (2.0 * B_full))
        nc.sync.dma_start(lossdb[:], sc[:])
        nc.gpsimd.collective_compute(
            "AllReduce", ALU.add,
            replica_groups=[list(range(NCORES))],
            ins=[lossdb.opt()], outs=[lossout.opt()])
        lf = sp.tile([1, 1], F32, tag="lf")
        nc.sync.dma_start(lf[:], lossout[:])
        nc.sync.dma_start(loss[:], lf[:])

    nc.compile()
    return nc


# ---------------------------------------------------------------------------
# host-side sharding
# ---------------------------------------------------------------------------

def shard_inputs(inputs, b, S, E, HD, T):
    KE, NH = E // 128, HD // 128
    bf = ml_dtypes.bfloat16
    f8 = ml_dtypes.float8_e4m3
    wdt = f8 if WHH_FP8 else bf
    x = np.asarray(inputs["x"], np.float32)
    labels = np.asarray(inputs["labels"]).astype(np.int32)
    trans = np.asarray(inputs["transitions"], np.float32)
    startv = np.asarray(inputs["start_trans"], np.float32).reshape(T, 1)
    endv = np.asarray(inputs["end_trans"], np.float32).reshape(T, 1)
    Wtag = np.asarray(inputs["W_tag"], np.float32)
    btag = np.asarray(inputs["b_tag"], np.float32).reshape(T, 1)
    iden = np.eye(128, dtype=np.float32).astype(wdt)

    per_dir = {}
    for d, sfx in enumerate(("f", "b")):
        Wih = np.asarray(inputs[f"W_ih_{sfx}"], np.float32)
        Whh = np.asarray(inputs[f"W_hh_{sfx}"], np.float32)
        bias = (np.asarray(inputs[f"b_ih_{sfx}"], np.float32)
                + np.asarray(inputs[f"b_hh_{sfx}"], np.float32))
        per_dir[d] = dict(
            wihT=np.ascontiguousarray(
                Wih.T.reshape(KE, 128, 4 * HD)).astype(bf),
            whhT=np.ascontiguousarray(
                (Whh * SCL).T.reshape(NH, 128, 4 * HD)).astype(wdt),
            bias4=np.ascontiguousarray(
                bias.reshape(4 * NH, 128).T).astype(np.float32),
            wtagT=np.ascontiguousarray(
                Wtag[:, d * HD:(d + 1) * HD].T.reshape(NH, 128, T)).astype(bf),
            tagb=btag if d == 0 else np.zeros_like(btag),
            m0=np.full((T, 1), 1.0 - d, np.float32),
            m1=np.full((T, 1), float(d), np.float32),
        )

    per_grp = {}
    for g in range(NPAIR):
        lab = labels[g * b:(g + 1) * b]      # (b, S)
        oh = (lab.T[None, :, :] == np.arange(T)[:, None, None])
        onehotI = np.ascontiguousarray(
            oh.reshape(T, S * b)).astype(np.float32)
        gold = (trans[lab[:, :-1], lab[:, 1:]].sum(1)
                + startv.reshape(-1)[lab[:, 0]]
                + endv.reshape(-1)[lab[:, -1]])
        per_grp[g] = (onehotI, gold.reshape(1, b).astype(np.float32))

    in_maps = []
    for c in range(NCORES):
        d = c // NPAIR                      # 0 fwd, 1 bwd
        g = c % NPAIR                       # batch group
        xs = x[g * b:(g + 1) * b]           # (b, S, E)
        if d == 1:
            xs = xs[:, ::-1, :]
        xTc = np.ascontiguousarray(xs.transpose(2, 1, 0).reshape(KE, 128, S * b)
                                   ).astype(bf)
        m = dict(per_dir[d])
        m["xT"] = xTc
        m["idenT"] = iden
        m["onehotI"] = per_grp[g][0]
        m["goldc"] = per_grp[g][1]
        m["transm"] = trans
        m["startv"] = startv
        m["endv"] = endv
        in_maps.append(m)
    return in_maps


# ---------------------------------------------------------------------------
# entry point
# ---------------------------------------------------------------------------

_B, _S, _E, _HD, _T = 64, 512, 1024, 512, 9
_cache = {}


def _get_program():
    if "nc" not in _cache:
        _cache["nc"] = build_program(_B // NPAIR, _S, _E, _HD, _T, _B)
    return _cache["nc"]


def kernel(**inputs) -> np.ndarray:
    from concourse.bass_utils import run_bass_kernel_spmd
    nc = _get_program()
    in_maps = shard_inputs(inputs, _B // NPAIR, _S, _E, _HD, _T)
    res = run_bass_kernel_spmd(nc, in_maps, list(range(NCORES)))
    out = np.asarray(res.results[0]["loss"], np.float32).reshape(())
    return out
